# revision 1
# baseline (speedup 1.0000x reference)
"""EnhancedGatedFusion fused kernel for 8 TRN2 NeuronCores.

Math (per token row x1, x2 of emb1/emb2; cat = [x1; x2], C = 2D):
  f_g = sigmoid(cat @ Wf[g].T + bf[g])          g = 0..2
  i_g = sigmoid(cat @ Wi[g].T + bi[g])
  u_g = tanh   (cat @ Wu[g].T + bu[g])
  gate_g = f_g * x1 + i_g * u_g
  fused = sum_g softmax(att_w)[g] * gate_g
  o = sigmoid(cat @ Wo.T + bo)
  out = LayerNorm(x1 + o * tanh(fused)) * gamma + beta

Sharding: data-parallel over the token axis (16384 / 8 = 2048 tokens per
core); the ten [D, C] weight blocks are replicated. Matmuls run in bf16
(fp32 PSUM accumulation); everything after the matmul is fp32.

Per-core device layout:
  xT [C, 2048] bf16 : cat^T shard (host-transposed).  lhsT tiles.
  wT [C, 10*D] bf16 : the ten weight blocks transposed, column order
                      f0,i0,u0,f1,i1,u1,f2,i2,u2,o.  rhs (streamed).
  bias matches wT column order, added into PSUM via a K=1 ones-matmul.
Loop nest: token-group (512 toks) -> D-chunk (512) -> quantity block ->
token-tile (128): 32 K-tile matmuls accumulate [128 tok, 512 dout] in
PSUM; ScalarE applies sigmoid/tanh on eviction; VectorE combines gates.
x = emb1 + output bounces through DRAM, then a LayerNorm pass
(bn_stats/bn_aggr over the free axis) produces the final rows.
"""

import sys

sys.path.insert(0, "/opt/trn_rl_repo")

import numpy as np
import ml_dtypes

import concourse.bass as bass
import concourse.tile as tile
from concourse import mybir, bacc
from concourse.bass_utils import run_bass_kernel_spmd

P = 128
N_CORES = 8
N_TOK, D_DIM, G_GATES = 16384, 2048, 3
C_DIM = 2 * D_DIM
NQ = 3 * G_GATES + 1  # f/i/u per gate + output gate
LN_EPS = 1e-5
BF16 = ml_dtypes.bfloat16


def _bcast_ap(vec: bass.AP, parts: int) -> bass.AP:
    """Partition-broadcast a 1-D DRAM vector to [parts, len]."""
    return bass.AP(tensor=vec.tensor, offset=vec.offset, ap=[[0, parts]] + vec.ap)


def build(
    n_cores: int,
    tokc: int,
    cdim: int,
    ddim: int,
    tg: int,
    dcq: int,
    w_soft,
    eps: float,
    reps: int = 1,
):
    """Build the per-core SPMD Bass program."""
    f32 = mybir.dt.float32
    bf = mybir.dt.bfloat16
    add, mult, sub = (
        mybir.AluOpType.add,
        mybir.AluOpType.mult,
        mybir.AluOpType.subtract,
    )
    AF = mybir.ActivationFunctionType

    nct = cdim // P  # K tiles
    nt = tg // P  # token tiles per group
    ngrp = tokc // tg  # token groups
    ndc = ddim // dcq  # D chunks
    nsub = ddim // 512  # bn_stats subgroups

    nc = bacc.Bacc("TRN2", target_bir_lowering=False, debug=False, num_devices=n_cores)
    xT = nc.dram_tensor("xT", [cdim, tokc], bf, kind="ExternalInput").ap()
    wT = nc.dram_tensor("wT", [cdim, NQ * ddim], bf, kind="ExternalInput").ap()
    bv = nc.dram_tensor("bv", [NQ * ddim], f32, kind="ExternalInput").ap()
    emb1 = nc.dram_tensor("emb1", [tokc, ddim], f32, kind="ExternalInput").ap()
    gamma = nc.dram_tensor("gamma", [ddim], f32, kind="ExternalInput").ap()
    beta = nc.dram_tensor("beta", [ddim], f32, kind="ExternalInput").ap()
    out = nc.dram_tensor("out", [tokc, ddim], f32, kind="ExternalOutput").ap()

    xT_r = xT.rearrange("(c p) n -> p c n", p=P)  # [128, nct, tokc]
    wT_r = wT.rearrange("(c p) n -> p c n", p=P)  # [128, nct, NQ*ddim]

    with tile.TileContext(nc) as tc:
        with (
            tc.tile_pool(name="const", bufs=1) as const,
            tc.tile_pool(name="xg", bufs=1) as xg_pool,
            tc.tile_pool(name="wq", bufs=2) as wq_pool,
            tc.tile_pool(name="bq", bufs=2) as bq_pool,
            tc.tile_pool(name="et", bufs=nt + 1) as et_pool,
            tc.tile_pool(name="fz", bufs=nt + 1) as fz_pool,
            tc.tile_pool(name="sf", bufs=nt) as sf_pool,
            tc.tile_pool(name="si", bufs=nt) as si_pool,
            tc.tile_pool(name="su", bufs=nt) as su_pool,
            tc.tile_pool(name="so", bufs=2) as so_pool,
            tc.tile_pool(name="tmp", bufs=2) as tmp_pool,
            tc.tile_pool(name="xf", bufs=1) as xf_pool,
            tc.tile_pool(name="st", bufs=4) as st_pool,
            tc.tile_pool(name="ps", bufs=6, space="PSUM") as ps_pool,
        ):
            gamma_t = const.tile([P, ddim], f32)
            nc.sync.dma_start(gamma_t[:], _bcast_ap(gamma, P))
            beta_t = const.tile([P, ddim], f32)
            nc.sync.dma_start(beta_t[:], _bcast_ap(beta, P))
            eps_t = const.tile([P, 1], f32)
            nc.vector.memset(eps_t[:], eps)

            for g in [g for _ in range(reps) for g in range(ngrp)]:
                xg = xg_pool.tile([P, nct, tg], bf)
                nc.sync.dma_start(xg[:], xT_r[:, :, g * tg : (g + 1) * tg])
                xf = xf_pool.tile([P, nt, ddim], f32)

                for dc in range(ndc):
                    dsl = slice(dc * dcq, (dc + 1) * dcq)
                    ets = []
                    for t in range(nt):
                        et = et_pool.tile([P, dcq], f32, tag="et")
                        r0 = g * tg + t * P
                        nc.sync.dma_start(et[:], emb1[r0 : r0 + P, dsl])
                        ets.append(et)
                    fz = [
                        fz_pool.tile([P, dcq], f32, tag="fz", name=f"fz{t}")
                        for t in range(nt)
                    ]
                    sfs, sis = None, None

                    for q in range(NQ):
                        col0 = q * ddim + dc * dcq
                        wq = wq_pool.tile([P, nct, dcq], bf)
                        nc.sync.dma_start(wq[:], wT_r[:, :, col0 : col0 + dcq])
                        bq = bq_pool.tile([P, dcq], f32)
                        nc.sync.dma_start(bq[:], _bcast_ap(bv[col0 : col0 + dcq], P))

                        kind = "o" if q == NQ - 1 else "fiu"[q % 3]
                        gate_i = q // 3
                        pool = {
                            "f": sf_pool,
                            "i": si_pool,
                            "u": su_pool,
                            "o": so_pool,
                        }[kind]
                        func = AF.Tanh if kind == "u" else AF.Sigmoid

                        stash = []
                        for t in range(nt):
                            ps = ps_pool.tile([P, dcq], f32)
                            for ci in range(nct):
                                nc.tensor.matmul(
                                    ps[:],
                                    lhsT=xg[:, ci, t * P : (t + 1) * P],
                                    rhs=wq[:, ci, :],
                                    start=(ci == 0),
                                    stop=(ci == nct - 1),
                                )
                            # bias on VectorE (PE stays pure matmul),
                            # bf16 pre-activation stash
                            s = pool.tile([P, dcq], bf, tag=kind)
                            nc.vector.tensor_add(s[:], ps[:], bq[:])
                            nc.scalar.activation(s[:], s[:], func)
                            stash.append(s)

                        if kind == "f":
                            sfs = stash
                        elif kind == "i":
                            sis = stash
                        elif kind == "u":
                            wgt = float(w_soft[gate_i])
                            for t in range(nt):
                                tA = tmp_pool.tile([P, dcq], f32, tag="tA")
                                nc.vector.tensor_mul(tA[:], sis[t][:], stash[t][:])
                                tB = tmp_pool.tile([P, dcq], f32, tag="tB")
                                nc.vector.tensor_mul(tB[:], sfs[t][:], ets[t][:])
                                nc.vector.tensor_add(tA[:], tA[:], tB[:])
                                if gate_i == 0:
                                    nc.vector.tensor_scalar_mul(fz[t][:], tA[:], wgt)
                                else:
                                    # fz += tA * w_g
                                    nc.vector.scalar_tensor_tensor(
                                        out=fz[t][:],
                                        in0=tA[:],
                                        scalar=wgt,
                                        in1=fz[t][:],
                                        op0=mult,
                                        op1=add,
                                    )
                        else:  # output gate: x = emb1 + o * tanh(fused)
                            for t in range(nt):
                                th = tmp_pool.tile([P, dcq], f32, tag="tA")
                                nc.scalar.activation(th[:], fz[t][:], AF.Tanh)
                                xc = tmp_pool.tile([P, dcq], f32, tag="tB")
                                nc.vector.tensor_mul(xc[:], stash[t][:], th[:])
                                nc.vector.tensor_add(
                                    xf[:, t, dsl], ets[t][:], xc[:]
                                )

                # LayerNorm on the resident group (in place, then DMA out)
                for t in range(nt):
                    xr = xf[:, t, :]
                    stats = st_pool.tile([P, nsub, 6], f32, tag="stats")
                    for s_i in range(nsub):
                        nc.vector.bn_stats(
                            stats[:, s_i, :], xr[:, s_i * 512 : (s_i + 1) * 512]
                        )
                    mv = st_pool.tile([P, 2], f32, tag="mv")
                    nc.vector.bn_aggr(mv[:], stats[:])
                    rstd = st_pool.tile([P, 1], f32, tag="rstd")
                    nc.scalar.activation(
                        rstd[:], mv[:, 1:2], AF.Sqrt, bias=eps_t[:], scale=1.0
                    )
                    nc.vector.reciprocal(rstd[:], rstd[:])
                    nc.vector.tensor_scalar(
                        out=xr[:],
                        in0=xr[:],
                        scalar1=mv[:, 0:1],
                        scalar2=rstd[:],
                        op0=sub,
                        op1=mult,
                    )
                    nc.vector.tensor_mul(xr[:], xr[:], gamma_t[:])
                    nc.vector.tensor_add(xr[:], xr[:], beta_t[:])
                    r0 = g * tg + t * P
                    nc.sync.dma_start(out[r0 : r0 + P, :], xr[:])
    nc.compile()
    return nc


def _prep_host(emb1, emb2, Wf, bfv, Wi, biv, Wu, buv, Wo, bov, att_w):
    """Host-side packing: softmax weights, transposed bf16 operands."""
    emb1 = np.asarray(emb1, dtype=np.float32)
    emb2 = np.asarray(emb2, dtype=np.float32)
    aw = np.asarray(att_w, dtype=np.float64)
    aw = np.exp(aw - aw.max())
    w_soft = (aw / aw.sum()).astype(np.float32)

    cols, bcols = [], []
    for gi in range(G_GATES):
        for W, b in ((Wf, bfv), (Wi, biv), (Wu, buv)):
            cols.append(np.asarray(W[gi], dtype=np.float32).T)
            bcols.append(np.asarray(b[gi], dtype=np.float32))
    cols.append(np.asarray(Wo, dtype=np.float32).T)
    bcols.append(np.asarray(bov, dtype=np.float32))
    wT = np.ascontiguousarray(np.concatenate(cols, axis=1)).astype(BF16)
    bv = np.concatenate(bcols).astype(np.float32)
    xT = np.ascontiguousarray(
        np.concatenate([emb1.T, emb2.T], axis=0).astype(BF16)
    )  # [C, N]
    return emb1, xT, wT, bv, w_soft


def kernel(emb1, emb2, Wf, bf, Wi, bi, Wu, bu, Wo, bo, att_w, gamma, beta):
    emb1, xT, wT, bv, w_soft = _prep_host(
        emb1, emb2, Wf, bf, Wi, bi, Wu, bu, Wo, bo, att_w
    )
    gamma = np.asarray(gamma, dtype=np.float32)
    beta = np.asarray(beta, dtype=np.float32)
    tokc = N_TOK // N_CORES

    nc = build(
        n_cores=N_CORES,
        tokc=tokc,
        cdim=C_DIM,
        ddim=D_DIM,
        tg=512,
        dcq=512,
        w_soft=w_soft,
        eps=LN_EPS,
    )
    in_maps = []
    for ci in range(N_CORES):
        s = slice(ci * tokc, (ci + 1) * tokc)
        in_maps.append(
            {
                "xT": np.ascontiguousarray(xT[:, s]),
                "wT": wT,
                "bv": bv,
                "emb1": np.ascontiguousarray(emb1[s]),
                "gamma": gamma,
                "beta": beta,
            }
        )
    res = run_bass_kernel_spmd(nc, in_maps, list(range(N_CORES)))
    return np.concatenate(
        [res.results[i]["out"] for i in range(N_CORES)], axis=0
    ).astype(np.float32)



# revision 4
# speedup vs baseline: 1.4906x; 1.4906x over previous
"""EnhancedGatedFusion fused kernel for 8 TRN2 NeuronCores (fp8 DoubleRow).

Math (per token row x1, x2 of emb1/emb2; cat = [x1; x2], C = 2D):
  f_g = sigmoid(cat @ Wf[g].T + bf[g])          g = 0..2
  i_g = sigmoid(cat @ Wi[g].T + bi[g])
  u_g = tanh   (cat @ Wu[g].T + bu[g])
  gate_g = f_g * x1 + i_g * u_g
  fused = sum_g softmax(att_w)[g] * gate_g
  o = sigmoid(cat @ Wo.T + bo)
  out = LayerNorm(x1 + o * tanh(fused)) * gamma + beta

Sharding: data-parallel over tokens (16384 / 8 = 2048 per core), weights
replicated.  The ten GEMMs run in fp8(e4m3) with MatmulPerfMode.DoubleRow
(two K-planes per PE pass -> 2x bf16 throughput); weights are pre-scaled
by 128 on the host so their magnitudes sit in e4m3's normal range, and
the 1/128 descale is folded into the activation's `scale` operand
(biases are host-scaled by 128 and added to PSUM before the activation).

Per-core schedule: 2 superblocks of 1024 tokens (8 tiles of 128).  Per
(sb, dchunk of 512 dout): stream each weight block [4096, 512] once
(block-contiguous fp8 DMA), run 16 DoubleRow matmuls per token tile into
a PSUM bank, evict via DVE (+bias) and Act (sigmoid/tanh with descale).
Gates combine on DVE into a bf16 fused accumulator; the o-gate stage
writes x = emb1 + o*tanh(fused) into a resident bf16 [128, 8, 2048]
tile and feeds bn_stats.  LayerNorm statistics aggregate across dchunks
(bn_aggr) and the apply + fp32 output DMA happen per token tile, which
overlaps the next superblock's matmuls.
"""

import sys

sys.path.insert(0, "/opt/trn_rl_repo")

import numpy as np
import ml_dtypes

import concourse.bass as bass
import concourse.tile as tile
from concourse import mybir, bacc
from concourse.bass_utils import run_bass_kernel_spmd

P = 128
N_CORES = 8
N_TOK, D_DIM, G_GATES = 16384, 2048, 3
C_DIM = 2 * D_DIM
NQ = 3 * G_GATES + 1  # f/i/u per gate + output gate
LN_EPS = 1e-5
BF16 = ml_dtypes.bfloat16
E4M3 = ml_dtypes.float8_e4m3
W_SCALE = 128.0  # host premultiplier for fp8 weights/biases
INV_W = 1.0 / W_SCALE


def _bcast_ap(vec: bass.AP, parts: int) -> bass.AP:
    """Partition-broadcast a 1-D DRAM vector to [parts, len]."""
    return bass.AP(tensor=vec.tensor, offset=vec.offset, ap=[[0, parts]] + vec.ap)


def build(n_cores, tokc, cdim, ddim, w_soft, eps, reps=1):
    """Per-core SPMD Bass program. tokc tokens/core, 2 superblocks."""
    f32 = mybir.dt.float32
    bf = mybir.dt.bfloat16
    f8 = mybir.dt.float8e4
    add, mult, sub = (
        mybir.AluOpType.add,
        mybir.AluOpType.mult,
        mybir.AluOpType.subtract,
    )
    AF = mybir.ActivationFunctionType
    DR = mybir.MatmulPerfMode.DoubleRow

    nct = cdim // P          # 32 K tiles of 128
    npair = nct // 2         # 16 DoubleRow pairs
    nsb = 2                  # superblocks
    sbt = tokc // nsb        # tokens per superblock (1024)
    nt = sbt // P            # token tiles per superblock (8)
    dcq = 512
    ndc = ddim // dcq        # 4 D chunks

    nc = bacc.Bacc("TRN2", target_bir_lowering=False, debug=False, num_devices=n_cores)
    xT = nc.dram_tensor("xT", [nsb, P, nct, sbt], f8, kind="ExternalInput").ap()
    wT = nc.dram_tensor("wT", [NQ, ndc, P, nct, dcq], f8, kind="ExternalInput").ap()
    bv = nc.dram_tensor("bv", [NQ * ndc, dcq], f32, kind="ExternalInput").ap()
    e1b = nc.dram_tensor("e1b", [tokc, ddim], bf, kind="ExternalInput").ap()
    gamma = nc.dram_tensor("gamma", [ddim], f32, kind="ExternalInput").ap()
    beta = nc.dram_tensor("beta", [ddim], f32, kind="ExternalInput").ap()
    out = nc.dram_tensor("out", [tokc, ddim], f32, kind="ExternalOutput").ap()

    from contextlib import ExitStack

    with tile.TileContext(nc) as tc, ExitStack() as es:
        pools = {
            "const": dict(bufs=1), "xg": dict(bufs=1), "wq": dict(bufs=2),
            "bq": dict(bufs=2), "et": dict(bufs=nt), "fz": dict(bufs=nt),
            "sf": dict(bufs=nt), "si": dict(bufs=nt), "su": dict(bufs=2),
            "so": dict(bufs=2), "ta": dict(bufs=2), "tb": dict(bufs=2),
            "th": dict(bufs=2), "xv": dict(bufs=1), "st": dict(bufs=nt),
            "mv": dict(bufs=4), "y": dict(bufs=2),
            "ps": dict(bufs=8, space="PSUM"),
        }
        pl = {
            name: es.enter_context(tc.tile_pool(name=name, **kw))
            for name, kw in pools.items()
        }
        const, xg_pool, wq_pool, bq_pool = pl["const"], pl["xg"], pl["wq"], pl["bq"]
        et_pool, fz_pool, sf_pool, si_pool = pl["et"], pl["fz"], pl["sf"], pl["si"]
        su_pool, so_pool, ta_pool, tb_pool = pl["su"], pl["so"], pl["ta"], pl["tb"]
        th_pool, xv_pool, st_pool, mv_pool = pl["th"], pl["xv"], pl["st"], pl["mv"]
        y_pool, ps_pool = pl["y"], pl["ps"]
        if True:
            gamma_t = const.tile([P, ddim], f32)
            nc.sync.dma_start(gamma_t[:], _bcast_ap(gamma, P))
            beta_t = const.tile([P, ddim], f32)
            nc.sync.dma_start(beta_t[:], _bcast_ap(beta, P))
            eps_t = const.tile([P, 1], f32)
            nc.vector.memset(eps_t[:], eps)

            for sb in [s for _ in range(reps) for s in range(nsb)]:
                xg = xg_pool.tile([P, nct, sbt], f8)
                nc.sync.dma_start(xg[:], xT[sb])
                xv = xv_pool.tile([P, nt, ddim], bf)
                stats = [
                    st_pool.tile([P, ndc, 6], f32, name=f"st{t}") for t in range(nt)
                ]

                for dc in range(ndc):
                    dsl = slice(dc * dcq, (dc + 1) * dcq)
                    ets = []
                    for t in range(nt):
                        et = et_pool.tile([P, dcq], bf, tag="et")
                        r0 = sb * sbt + t * P
                        nc.sync.dma_start(et[:], e1b[r0 : r0 + P, dsl])
                        ets.append(et)
                    fz = [
                        fz_pool.tile([P, dcq], bf, tag="fz", name=f"fz{t}")
                        for t in range(nt)
                    ]
                    sfs, sis = None, None

                    for q in range(NQ):
                        wq = wq_pool.tile([P, nct, dcq], f8)
                        nc.sync.dma_start(wq[:], wT[q][dc])
                        bq = bq_pool.tile([P, dcq], f32)
                        nc.sync.dma_start(
                            bq[:], _bcast_ap(bv[q * ndc + dc], P)
                        )

                        kind = "o" if q == NQ - 1 else "fiu"[q % 3]
                        gate_i = q // 3
                        pool = {
                            "f": sf_pool,
                            "i": si_pool,
                            "u": su_pool,
                            "o": so_pool,
                        }[kind]
                        func = AF.Tanh if kind == "u" else AF.Sigmoid

                        stash = []
                        for t in range(nt):
                            ps = ps_pool.tile([P, dcq], f32)
                            for ci in range(npair):
                                nc.tensor.matmul(
                                    ps[:],
                                    lhsT=xg[:, 2 * ci : 2 * ci + 2, t * P : (t + 1) * P],
                                    rhs=wq[:, 2 * ci : 2 * ci + 2, :],
                                    start=(ci == 0),
                                    stop=(ci == npair - 1),
                                    perf_mode=DR,
                                )
                            s = pool.tile([P, dcq], bf, tag=kind)
                            nc.vector.tensor_add(s[:], ps[:], bq[:])
                            # func((psum + 128*b) / 128) on the Act engine
                            nc.scalar.activation(s[:], s[:], func, scale=INV_W)
                            stash.append(s)

                            if kind == "u":
                                wgt = float(w_soft[gate_i])
                                tA = ta_pool.tile([P, dcq], f32, tag="tA")
                                nc.vector.tensor_mul(tA[:], sis[t][:], s[:])
                                tB = tb_pool.tile([P, dcq], f32, tag="tB")
                                nc.vector.tensor_mul(tB[:], sfs[t][:], ets[t][:])
                                nc.vector.tensor_add(tA[:], tA[:], tB[:])
                                if gate_i == 0:
                                    nc.vector.tensor_scalar_mul(fz[t][:], tA[:], wgt)
                                else:
                                    nc.vector.scalar_tensor_tensor(
                                        out=fz[t][:],
                                        in0=tA[:],
                                        scalar=wgt,
                                        in1=fz[t][:],
                                        op0=mult,
                                        op1=add,
                                    )
                            elif kind == "o":
                                th = th_pool.tile([P, dcq], f32, tag="th")
                                nc.scalar.activation(th[:], fz[t][:], AF.Tanh)
                                xc = ta_pool.tile([P, dcq], f32, tag="tA")
                                nc.vector.tensor_mul(xc[:], s[:], th[:])
                                nc.vector.tensor_add(
                                    xv[:, t, dsl], ets[t][:], xc[:]
                                )
                                nc.vector.bn_stats(
                                    stats[t][:, dc, :], xv[:, t, dsl]
                                )

                        if kind == "f":
                            sfs = stash
                        elif kind == "i":
                            sis = stash

                # LayerNorm per token tile (overlaps next superblock's matmuls)
                for t in range(nt):
                    mv = mv_pool.tile([P, 2], f32, tag="mv")
                    nc.vector.bn_aggr(mv[:], stats[t][:])
                    rstd = mv_pool.tile([P, 1], f32, tag="rstd")
                    nc.scalar.activation(
                        rstd[:], mv[:, 1:2], AF.Sqrt, bias=eps_t[:], scale=1.0
                    )
                    nc.vector.reciprocal(rstd[:], rstd[:])
                    y = y_pool.tile([P, ddim], f32)
                    nc.vector.tensor_scalar(
                        out=y[:],
                        in0=xv[:, t, :],
                        scalar1=mv[:, 0:1],
                        scalar2=rstd[:],
                        op0=sub,
                        op1=mult,
                    )
                    nc.vector.tensor_mul(y[:], y[:], gamma_t[:])
                    nc.vector.tensor_add(y[:], y[:], beta_t[:])
                    r0 = sb * sbt + t * P
                    nc.sync.dma_start(out[r0 : r0 + P, :], y[:])
    nc.compile()
    return nc


def _prep_host(emb1, emb2, Wf, bfv, Wi, biv, Wu, buv, Wo, bov, att_w):
    """Host-side packing: softmax weights, fp8 transposed operands."""
    emb1 = np.asarray(emb1, dtype=np.float32)
    emb2 = np.asarray(emb2, dtype=np.float32)
    aw = np.asarray(att_w, dtype=np.float64)
    aw = np.exp(aw - aw.max())
    w_soft = (aw / aw.sum()).astype(np.float32)

    cols, bcols = [], []
    for gi in range(G_GATES):
        for W, b in ((Wf, bfv), (Wi, biv), (Wu, buv)):
            cols.append(np.asarray(W[gi], dtype=np.float32).T)
            bcols.append(np.asarray(b[gi], dtype=np.float32))
    cols.append(np.asarray(Wo, dtype=np.float32).T)
    bcols.append(np.asarray(bov, dtype=np.float32))
    wcat = np.concatenate(cols, axis=1) * W_SCALE  # [C, NQ*D]
    # [NQ, ndc, P, nct, dcq] block-contiguous fp8 layout
    nct, ndc, dcq = C_DIM // P, D_DIM // 512, 512
    wTs = np.ascontiguousarray(
        wcat.reshape(nct, P, NQ, ndc, dcq).transpose(2, 3, 1, 0, 4)
    ).astype(E4M3)
    # biases scaled by W_SCALE, grouped [NQ*ndc, dcq]
    bvs = (np.concatenate(bcols).reshape(NQ, ndc, dcq) * W_SCALE).reshape(
        NQ * ndc, dcq
    ).astype(np.float32)

    xT = np.concatenate([emb1.T, emb2.T], axis=0).astype(E4M3)  # [C, N]
    e1b = emb1.astype(BF16)
    return xT, wTs, bvs, e1b, w_soft


def _pack_x(xT, s, tokc):
    """Per-core [nsb, P, nct, sbt] block-contiguous fp8 token shard."""
    nct, nsb = C_DIM // P, 2
    sbt = tokc // nsb
    xs = xT[:, s]  # [C, tokc]
    return np.ascontiguousarray(
        xs.reshape(nct, P, nsb, sbt).transpose(2, 1, 0, 3)
    )


def kernel(emb1, emb2, Wf, bf, Wi, bi, Wu, bu, Wo, bo, att_w, gamma, beta):
    xT, wTs, bvs, e1b, w_soft = _prep_host(
        emb1, emb2, Wf, bf, Wi, bi, Wu, bu, Wo, bo, att_w
    )
    gamma = np.asarray(gamma, dtype=np.float32)
    beta = np.asarray(beta, dtype=np.float32)
    tokc = N_TOK // N_CORES

    nc = build(
        n_cores=N_CORES,
        tokc=tokc,
        cdim=C_DIM,
        ddim=D_DIM,
        w_soft=w_soft,
        eps=LN_EPS,
    )
    in_maps = []
    for ci in range(N_CORES):
        s = slice(ci * tokc, (ci + 1) * tokc)
        in_maps.append(
            {
                "xT": _pack_x(xT, s, tokc),
                "wT": wTs,
                "bv": bvs,
                "e1b": np.ascontiguousarray(e1b[s]),
                "gamma": gamma,
                "beta": beta,
            }
        )
    res = run_bass_kernel_spmd(nc, in_maps, list(range(N_CORES)))
    return np.concatenate(
        [res.results[i]["out"] for i in range(N_CORES)], axis=0
    ).astype(np.float32)
